# revision 3
# baseline (speedup 1.0000x reference)
"""BERT input representation kernel for 8 TRN2 NeuronCores.

Math (reference):
    x1  = x @ W_emb + b_emb                      # [B,S,D]
    seg = einsum('bnsd,s->bnd', x1.reshape(B,S/8,8,D), w_seg) + b_seg
    out = (x1.reshape(...) + seg[:,:,None,:]).reshape(B,S,D) + PE(S,D)

Folded form used here (exact algebra):
    out[b,s,:] = (A @ x[b])[s,:] @ W_emb + bias[s,:]
where A = I + blockdiag(ones(8,1) @ w_seg[None,:]) mixes rows within each
8-row segment, and bias[s,:] = PE[s,:] + b_emb*(1 + sum(w_seg)) + b_seg.

Sharding: pure data-parallel over batch; each of 8 cores handles 8
batches (4096 rows = 32 row-tiles of 128 rows = 16 tile-pair groups).

v2 schedule: the epilogue (PSUM drain + bias add) was the bottleneck in
v1 (64 DVE tensor_tensor ops at 1x mode = 46 us serial).  Now:
  - output is stored bf16 (host upcasts to f32) -> store traffic halves
  - per pair, one [128, 2048] f32 PSUM tile (4 banks; 2 bufs = all 8):
    the transpose+segment-mix matmul writes cols 0:128, ACT copies it
    out to resident bf16 x~^T, then 5 main matmuls fill the tile
  - epilogue split across engines: DVE does a fused drain+bias
    tensor_tensor on cols 0:VC; ACT pre-fills bias into cols VC:2048
    (ScalarE->PSUM write) before the start=False mains accumulate onto
    it, so ACT's drain is a plain copy
  - two 256 KiB stores per pair on the sync HWDGE ring (first store
    gates on DVE only)
"""

import sys

if "/opt/trn_rl_repo" not in sys.path:
    sys.path.insert(0, "/opt/trn_rl_repo")

import ml_dtypes
import numpy as np

import concourse.bacc as bacc
import concourse.mybir as mybir
import concourse.tile as tile
from concourse.bass_utils import run_bass_kernel_spmd

B, S, F, D, SEG = 64, 512, 64, 1024, 8
N_CORES = 8
B_LOC = B // N_CORES          # batches per core
ROWS = B_LOC * S              # 4096 rows per core
TILE_P = 128                  # rows per tile
N_TILES = ROWS // TILE_P      # 32
N_PAIR = N_TILES // 2         # 16 tile-pairs
N_BIAS = S // TILE_P          # 4 distinct bias row-tiles
PW = 2 * D                    # 2048 cols per pair psum tile
VC = 1280                     # DVE fused drain+bias covers cols [0:VC)

_NC_CACHE = None


def _build_nc():
    nc = bacc.Bacc("TRN2", target_bir_lowering=False, debug=False,
                   num_devices=N_CORES)
    # x pre-rearranged on host (layout + cast to bf16):
    # xr[p, i*F:(i+1)*F] = x[i*128+p]; cols [0:128] = A^T
    x_d = nc.declare_dram_parameter("x", [TILE_P, TILE_P + N_TILES * F],
                                    mybir.dt.bfloat16, isOutput=False)
    # combined constants [128, 5120]: cols [0:1024]=W stacked twice
    # (partitions 0-63 and 64-127 both hold W) | [1024:5120]=bias0..3
    cc_d = nc.declare_dram_parameter("cc", [TILE_P, 5 * D],
                                     mybir.dt.bfloat16, isOutput=False)
    out_d = nc.declare_dram_parameter("out", [ROWS, D], mybir.dt.bfloat16,
                                      isOutput=True)

    with tile.TileContext(nc) as tc:
        with (
            tc.tile_pool(name="const", bufs=1) as cpool,
            tc.tile_pool(name="xbf", bufs=2) as xbpool,
            tc.tile_pool(name="outp", bufs=4) as opool,
            tc.tile_pool(name="ps", bufs=2, space="PSUM") as psp,
        ):
            # first-wave loads: A^T + pair-0 x on sync ring, W on scalar
            # ring, bias0/1 on sync behind A^T, bias2/3 + later x waves on
            # scalar behind W.
            at_x0 = cpool.tile([TILE_P, 2 * TILE_P], mybir.dt.bfloat16)
            nc.sync.dma_start(at_x0[:], x_d[:, 0:2 * TILE_P])
            at_ap = at_x0[:, 0:TILE_P]
            cc_sb = cpool.tile([TILE_P, 5 * D], mybir.dt.bfloat16)
            nc.scalar.dma_start(cc_sb[:, 0:D], cc_d[:, 0:D])
            nc.sync.dma_start(cc_sb[:, D:3 * D], cc_d[:, D:3 * D])

            def bias_pair(j):
                base = D + ((2 * j) % N_BIAS) * D
                return cc_sb[:, base:base + PW]

            def w_ap(u, lo, hi):
                return cc_sb[64 * u:64 * u + F, lo:hi]

            # resident x~^T (bf16): xt_sb[64u+f, 128j+n] = x~[2j+u, n, f]
            xt_sb = cpool.tile([TILE_P, N_PAIR * TILE_P], mybir.dt.bfloat16)

            WAVES = [1, 1, 2, 4, 4, 4]
            pr0 = 0
            for wn, wp in enumerate(WAVES):
                c0, cw = pr0 * TILE_P, wp * TILE_P   # x cols of this wave
                if wn == 0:
                    xcb = at_x0[:, TILE_P:2 * TILE_P]
                else:
                    xcb = xbpool.tile([TILE_P, 512], mybir.dt.bfloat16,
                                      name="xcb", tag="xcb")
                    nc.scalar.dma_start(xcb[:, 0:cw],
                                        x_d[:, TILE_P + c0:TILE_P + c0 + cw])
                if wn == 1:
                    nc.scalar.dma_start(cc_sb[:, 3 * D:5 * D],
                                        cc_d[:, 3 * D:5 * D])

                for k in range(wp):
                    j = pr0 + k
                    bias = bias_pair(j)
                    pair = psp.tile([TILE_P, PW], mybir.dt.float32,
                                    name="pair", tag="pair")
                    # transpose + segment mix for both tiles of the pair
                    nc.tensor.matmul(pair[:, 0:TILE_P],
                                     xcb[:, 128 * k:128 * (k + 1)],
                                     at_ap, start=True, stop=True)
                    xt = xt_sb[:, 128 * j:128 * (j + 1)]
                    nc.scalar.copy(xt, pair[:, 0:TILE_P])
                    # ACT pre-fills bias into cols [D:PW) — bank-aligned
                    # (a start=True matmul resets its whole PSUM bank, so
                    # prefilled and start=True regions can't share a bank)
                    nc.scalar.copy(pair[:, D:PW], bias[:, D:PW])
                    lhs0 = xt_sb[0:64, 128 * j:128 * (j + 1)]
                    lhs1 = xt_sb[64:128, 128 * j:128 * (j + 1)]
                    # mains: tile 2j on cols [0:1024) (start=True), tile
                    # 2j+1 on [1024:2048) accumulating onto the bias
                    nc.tensor.matmul(pair[:, 0:512], lhs0,
                                     w_ap(0, 0, 512), start=True, stop=True)
                    nc.tensor.matmul(pair[:, 512:1024], lhs0,
                                     w_ap(0, 512, 1024),
                                     start=True, stop=True)
                    nc.tensor.matmul(pair[:, 1024:1536], lhs1,
                                     w_ap(1, 0, 512),
                                     start=False, stop=True,
                                     skip_group_check=True)
                    nc.tensor.matmul(pair[:, 1536:2048], lhs1,
                                     w_ap(1, 512, 1024),
                                     start=False, stop=True,
                                     skip_group_check=True)
                    o_t = opool.tile([TILE_P, PW], mybir.dt.bfloat16,
                                     name="o_t")
                    # DVE: fused drain+bias on [0:D), plain drain [D:VC)
                    nc.vector.tensor_add(o_t[:, 0:D], pair[:, 0:D],
                                         bias[:, 0:D])
                    nc.vector.tensor_copy(o_t[:, D:VC], pair[:, D:VC])
                    # ACT drain of the rest (bias already accumulated)
                    nc.scalar.copy(o_t[:, VC:PW], pair[:, VC:PW])
                    rows0 = out_d[256 * j:256 * j + 128, :]
                    rows1 = out_d[256 * j + 128:256 * j + 256, :]
                    nc.sync.dma_start(rows0, o_t[:, 0:D])
                    nc.sync.dma_start(rows1, o_t[:, D:PW])
                pr0 += wp
    nc.compile()
    return nc


def _host_constants(W_emb, b_emb, w_seg, b_seg):
    # sinusoidal positional encoding, float32, same formula as the reference
    pos = np.arange(S, dtype=np.float32)[:, None]
    div = np.exp(np.arange(0, D, 2, dtype=np.float32)
                 * (-np.log(10000.0) / D)).astype(np.float32)
    ang = pos * div
    pe = np.zeros((S, D), np.float32)
    pe[:, 0::2] = np.sin(ang)
    pe[:, 1::2] = np.cos(ang)

    bias = (pe + b_emb[None, :] * (np.float32(1.0) + w_seg.sum())
            + b_seg[0]).astype(np.float32)
    # rearrange to [128, 4*D]: column block j holds bias rows j*128..j*128+127
    bias_r = np.ascontiguousarray(
        bias.reshape(N_BIAS, TILE_P, D).transpose(1, 0, 2).reshape(
            TILE_P, N_BIAS * D)).astype(ml_dtypes.bfloat16)

    blk = np.eye(SEG, dtype=np.float32) + w_seg[:, None] * np.ones(
        (1, SEG), np.float32)
    at = np.kron(np.eye(TILE_P // SEG, dtype=np.float32), blk).astype(
        ml_dtypes.bfloat16)

    wb = np.vstack([W_emb, W_emb]).astype(ml_dtypes.bfloat16)
    # combined consts: [W2|bias0|bias1|bias2|bias3] as [128, 5*D] bf16
    cc = np.ascontiguousarray(np.concatenate([wb, bias_r], axis=1))
    return at, cc


def _prepare_in_maps(x, W_emb, b_emb, w_seg, b_seg):
    x = np.ascontiguousarray(np.asarray(x, dtype=np.float32))
    W_emb = np.asarray(W_emb, dtype=np.float32)
    b_emb = np.asarray(b_emb, dtype=np.float32)
    w_seg = np.asarray(w_seg, dtype=np.float32)
    b_seg = np.asarray(b_seg, dtype=np.float32)

    at, cc = _host_constants(W_emb, b_emb, w_seg, b_seg)

    in_maps = []
    for c in range(N_CORES):
        xs = x[c * B_LOC:(c + 1) * B_LOC].reshape(ROWS, F)
        # rearrange [32 tiles, 128 rows, F] -> [128, 32*F], bf16 staging
        xr = np.ascontiguousarray(
            xs.reshape(N_TILES, TILE_P, F).transpose(1, 0, 2).reshape(
                TILE_P, N_TILES * F)).astype(ml_dtypes.bfloat16)
        in_maps.append(
            {"x": np.ascontiguousarray(np.concatenate([at, xr], axis=1)),
             "cc": cc})
    return in_maps


def kernel(x, W_emb, b_emb, w_seg, b_seg):
    in_maps = _prepare_in_maps(x, W_emb, b_emb, w_seg, b_seg)

    global _NC_CACHE
    if _NC_CACHE is None:
        _NC_CACHE = _build_nc()

    res = run_bass_kernel_spmd(_NC_CACHE, in_maps,
                               core_ids=list(range(N_CORES)))
    out = np.concatenate(
        [np.asarray(res.results[c]["out"]).astype(np.float32).reshape(
            B_LOC, S, D) for c in range(N_CORES)], axis=0)
    return out


# revision 5
# speedup vs baseline: 1.4865x; 1.4865x over previous
"""BERT input representation kernel for 8 TRN2 NeuronCores.

Math (reference):
    x1  = x @ W_emb + b_emb                      # [B,S,D]
    seg = einsum('bnsd,s->bnd', x1.reshape(B,S/8,8,D), w_seg) + b_seg
    out = (x1.reshape(...) + seg[:,:,None,:]).reshape(B,S,D) + PE(S,D)

Folded form used here (exact algebra):
    out[b,s,:] = (A @ x[b])[s,:] @ W_emb + bias[s,:]
where A = I + blockdiag(ones(8,1) @ w_seg[None,:]) mixes rows within each
8-row segment, and bias[s,:] = PE[s,:] + b_emb*(1 + sum(w_seg)) + b_seg.

Sharding: pure data-parallel over batch; each of 8 cores handles 8
batches (4096 rows = 32 row-tiles of 128 rows = 16 tile-pair groups).

v3 schedule:
  - output stored bf16 (host upcasts to f32): store traffic halves
  - prologue: all of x loads in 3 DMAs; all 16 transpose+segment-mix
    matmuls run into one 4-bank PSUM workspace; 4 big ACT copies build
    the resident bf16 x~^T.  The steady loop then has no PE<->ACT
    ping-pong.
  - per pair j: one [128,2048] f32 PSUM tile (4 banks, 2 bufs = all 8),
    4 mains (start=True, FD=512).  Epilogue split: DVE fused
    drain+bias tensor_tensor on cols [0:XV) (PSUM 1x mode), ACT
    plain-drains [XV:2048) (1x), DVE then adds bias there as a bf16
    SBUF tensor_tensor (2x packed mode).  The DVE add for pair j is
    emitted after pair j+1's fused op (software pipelining) so DVE
    never idles waiting for ACT.
  - two 256 KiB bf16 stores per pair on the sync HWDGE ring
"""

import sys

if "/opt/trn_rl_repo" not in sys.path:
    sys.path.insert(0, "/opt/trn_rl_repo")

import ml_dtypes
import numpy as np

import concourse.bacc as bacc
import concourse.mybir as mybir
import concourse.tile as tile
from concourse.bass_utils import run_bass_kernel_spmd

B, S, F, D, SEG = 64, 512, 64, 1024, 8
N_CORES = 8
B_LOC = B // N_CORES          # batches per core
ROWS = B_LOC * S              # 4096 rows per core
TILE_P = 128                  # rows per tile
N_TILES = ROWS // TILE_P      # 32
N_PAIR = N_TILES // 2         # 16 tile-pairs
N_BIAS = S // TILE_P          # 4 distinct bias row-tiles
PW = 2 * D                    # 2048 cols per pair psum tile
XV = 448                      # DVE fused drain+bias covers cols [0:XV)

_NC_CACHE = None


def _build_nc():
    nc = bacc.Bacc("TRN2", target_bir_lowering=False, debug=False,
                   num_devices=N_CORES)
    # x pre-rearranged on host (layout + cast to bf16):
    # xr[p, i*F:(i+1)*F] = x[i*128+p]; cols [0:128] = A^T
    x_d = nc.declare_dram_parameter("x", [TILE_P, TILE_P + N_TILES * F],
                                    mybir.dt.bfloat16, isOutput=False)
    # combined constants [128, 5120]: cols [0:1024]=W stacked twice
    # (partitions 0-63 and 64-127 both hold W) | [1024:5120]=bias0..3
    cc_d = nc.declare_dram_parameter("cc", [TILE_P, 5 * D],
                                     mybir.dt.bfloat16, isOutput=False)
    out_d = nc.declare_dram_parameter("out", [ROWS, D], mybir.dt.bfloat16,
                                      isOutput=True)

    with tile.TileContext(nc) as tc:
        with (
            tc.tile_pool(name="const", bufs=1) as cpool,
            tc.tile_pool(name="outp", bufs=4) as opool,
            tc.tile_pool(name="ps", bufs=2, space="PSUM") as psp,
        ):
            # loads: sync ring carries A^T+x then the stores; scalar ring
            # carries W and the bias tiles in need-order.
            at_x0 = cpool.tile([TILE_P, 2 * TILE_P], mybir.dt.bfloat16)
            nc.sync.dma_start(at_x0[:], x_d[:, 0:2 * TILE_P])
            at_ap = at_x0[:, 0:TILE_P]
            cc_sb = cpool.tile([TILE_P, 5 * D], mybir.dt.bfloat16)
            nc.scalar.dma_start(cc_sb[:, 0:D], cc_d[:, 0:D])
            xr_sb = cpool.tile([TILE_P, (N_PAIR - 1) * TILE_P],
                               mybir.dt.bfloat16)
            nc.sync.dma_start(xr_sb[:, 0:7 * TILE_P],
                              x_d[:, 2 * TILE_P:9 * TILE_P])
            nc.scalar.dma_start(cc_sb[:, D:3 * D], cc_d[:, D:3 * D])
            nc.sync.dma_start(xr_sb[:, 7 * TILE_P:15 * TILE_P],
                              x_d[:, 9 * TILE_P:17 * TILE_P])
            nc.scalar.dma_start(cc_sb[:, 3 * D:5 * D], cc_d[:, 3 * D:5 * D])

            def bias_pair(j):
                base = D + ((2 * j) % N_BIAS) * D
                return cc_sb[:, base:base + PW]

            def w_ap(u, lo, hi):
                return cc_sb[64 * u:64 * u + F, lo:hi]

            # resident x~^T (bf16): xt_sb[64u+f, 128j+n] = x~[2j+u, n, f]
            xt_sb = cpool.tile([TILE_P, N_PAIR * TILE_P], mybir.dt.bfloat16)

            # prologue: all transpose+mix matmuls into one 4-bank psum
            # workspace; one ACT copy per 4 pairs builds xt_sb.
            ws = psp.tile([TILE_P, PW], mybir.dt.float32, name="ws",
                          tag="pair")
            for b4 in range(4):
                for k in range(4):
                    pr = 4 * b4 + k
                    src = (at_x0[:, TILE_P:2 * TILE_P] if pr == 0 else
                           xr_sb[:, 128 * (pr - 1):128 * pr])
                    nc.tensor.matmul(ws[:, 512 * b4 + 128 * k:
                                        512 * b4 + 128 * (k + 1)],
                                     src, at_ap, start=True, stop=True)
                nc.scalar.copy(xt_sb[:, 512 * b4:512 * (b4 + 1)],
                               ws[:, 512 * b4:512 * (b4 + 1)])

            # steady loop, software-pipelined by one pair on DVE
            prev = None
            for j in range(N_PAIR):
                bias = bias_pair(j)
                pair = psp.tile([TILE_P, PW], mybir.dt.float32,
                                name="pair", tag="pair")
                lhs0 = xt_sb[0:64, 128 * j:128 * (j + 1)]
                lhs1 = xt_sb[64:128, 128 * j:128 * (j + 1)]
                nc.tensor.matmul(pair[:, 0:512], lhs0,
                                 w_ap(0, 0, 512), start=True, stop=True)
                nc.tensor.matmul(pair[:, 512:1024], lhs0,
                                 w_ap(0, 512, 1024), start=True, stop=True)
                nc.tensor.matmul(pair[:, 1024:1536], lhs1,
                                 w_ap(1, 0, 512), start=True, stop=True)
                nc.tensor.matmul(pair[:, 1536:2048], lhs1,
                                 w_ap(1, 512, 1024), start=True, stop=True)
                o_t = opool.tile([TILE_P, PW], mybir.dt.bfloat16,
                                 name="o_t")
                # DVE fused drain+bias (PSUM 1x), ACT plain drain (1x)
                nc.vector.tensor_add(o_t[:, 0:XV], pair[:, 0:XV],
                                     bias[:, 0:XV])
                nc.scalar.copy(o_t[:, XV:PW], pair[:, XV:PW])
                if prev is not None:
                    _finish_pair(nc, out_d, *prev)
                prev = (j, o_t, bias)
            _finish_pair(nc, out_d, *prev)
    nc.compile()
    return nc


def _finish_pair(nc, out_d, j, o_t, bias):
    # bias add for ACT's drained region: bf16 SBUF tensor_tensor (2x),
    # in place; then both stores.
    nc.vector.tensor_add(o_t[:, XV:PW], o_t[:, XV:PW], bias[:, XV:PW])
    nc.sync.dma_start(out_d[256 * j:256 * j + 128, :], o_t[:, 0:D])
    nc.sync.dma_start(out_d[256 * j + 128:256 * j + 256, :], o_t[:, D:PW])


def _host_constants(W_emb, b_emb, w_seg, b_seg):
    # sinusoidal positional encoding, float32, same formula as the reference
    pos = np.arange(S, dtype=np.float32)[:, None]
    div = np.exp(np.arange(0, D, 2, dtype=np.float32)
                 * (-np.log(10000.0) / D)).astype(np.float32)
    ang = pos * div
    pe = np.zeros((S, D), np.float32)
    pe[:, 0::2] = np.sin(ang)
    pe[:, 1::2] = np.cos(ang)

    bias = (pe + b_emb[None, :] * (np.float32(1.0) + w_seg.sum())
            + b_seg[0]).astype(np.float32)
    # rearrange to [128, 4*D]: column block j holds bias rows j*128..j*128+127
    bias_r = np.ascontiguousarray(
        bias.reshape(N_BIAS, TILE_P, D).transpose(1, 0, 2).reshape(
            TILE_P, N_BIAS * D)).astype(ml_dtypes.bfloat16)

    blk = np.eye(SEG, dtype=np.float32) + w_seg[:, None] * np.ones(
        (1, SEG), np.float32)
    at = np.kron(np.eye(TILE_P // SEG, dtype=np.float32), blk).astype(
        ml_dtypes.bfloat16)

    wb = np.vstack([W_emb, W_emb]).astype(ml_dtypes.bfloat16)
    # combined consts: [W2|bias0|bias1|bias2|bias3] as [128, 5*D] bf16
    cc = np.ascontiguousarray(np.concatenate([wb, bias_r], axis=1))
    return at, cc


def _prepare_in_maps(x, W_emb, b_emb, w_seg, b_seg):
    x = np.ascontiguousarray(np.asarray(x, dtype=np.float32))
    W_emb = np.asarray(W_emb, dtype=np.float32)
    b_emb = np.asarray(b_emb, dtype=np.float32)
    w_seg = np.asarray(w_seg, dtype=np.float32)
    b_seg = np.asarray(b_seg, dtype=np.float32)

    at, cc = _host_constants(W_emb, b_emb, w_seg, b_seg)

    in_maps = []
    for c in range(N_CORES):
        xs = x[c * B_LOC:(c + 1) * B_LOC].reshape(ROWS, F)
        # rearrange [32 tiles, 128 rows, F] -> [128, 32*F], bf16 staging
        xr = np.ascontiguousarray(
            xs.reshape(N_TILES, TILE_P, F).transpose(1, 0, 2).reshape(
                TILE_P, N_TILES * F)).astype(ml_dtypes.bfloat16)
        in_maps.append(
            {"x": np.ascontiguousarray(np.concatenate([at, xr], axis=1)),
             "cc": cc})
    return in_maps


def kernel(x, W_emb, b_emb, w_seg, b_seg):
    in_maps = _prepare_in_maps(x, W_emb, b_emb, w_seg, b_seg)

    global _NC_CACHE
    if _NC_CACHE is None:
        _NC_CACHE = _build_nc()

    res = run_bass_kernel_spmd(_NC_CACHE, in_maps,
                               core_ids=list(range(N_CORES)))
    out = np.concatenate(
        [np.asarray(res.results[c]["out"]).astype(np.float32).reshape(
            B_LOC, S, D) for c in range(N_CORES)], axis=0)
    return out


# revision 7
# speedup vs baseline: 1.4968x; 1.0069x over previous
"""BERT input representation kernel for 8 TRN2 NeuronCores.

Math (reference):
    x1  = x @ W_emb + b_emb                      # [B,S,D]
    seg = einsum('bnsd,s->bnd', x1.reshape(B,S/8,8,D), w_seg) + b_seg
    out = (x1.reshape(...) + seg[:,:,None,:]).reshape(B,S,D) + PE(S,D)

Folded form used here (exact algebra):
    out[b,s,:] = (A @ x[b])[s,:] @ W_emb + bias[s,:]
where A = I + blockdiag(ones(8,1) @ w_seg[None,:]) mixes rows within each
8-row segment, and bias[s,:] = PE[s,:] + b_emb*(1 + sum(w_seg)) + b_seg.

Sharding: pure data-parallel over batch; each of 8 cores handles 8
batches (4096 rows = 32 row-tiles of 128 rows = 16 tile-pair groups).

v3 schedule:
  - output stored bf16 (host upcasts to f32): store traffic halves
  - prologue: all of x loads in 3 DMAs; all 16 transpose+segment-mix
    matmuls run into one 4-bank PSUM workspace; 4 big ACT copies build
    the resident bf16 x~^T.  The steady loop then has no PE<->ACT
    ping-pong.
  - per pair j: one [128,2048] f32 PSUM tile (4 banks, 2 bufs = all 8),
    4 mains (start=True, FD=512).  Epilogue split: DVE fused
    drain+bias tensor_tensor on cols [0:XV) (PSUM 1x mode), ACT
    plain-drains [XV:2048) (1x), DVE then adds bias there as a bf16
    SBUF tensor_tensor (2x packed mode).  The DVE add for pair j is
    emitted after pair j+1's fused op (software pipelining) so DVE
    never idles waiting for ACT.
  - two 256 KiB bf16 stores per pair on the sync HWDGE ring
"""

import sys

if "/opt/trn_rl_repo" not in sys.path:
    sys.path.insert(0, "/opt/trn_rl_repo")

import ml_dtypes
import numpy as np

import concourse.bacc as bacc
import concourse.mybir as mybir
import concourse.tile as tile
from concourse.bass_utils import run_bass_kernel_spmd

B, S, F, D, SEG = 64, 512, 64, 1024, 8
N_CORES = 8
B_LOC = B // N_CORES          # batches per core
ROWS = B_LOC * S              # 4096 rows per core
TILE_P = 128                  # rows per tile
N_TILES = ROWS // TILE_P      # 32
N_PAIR = N_TILES // 2         # 16 tile-pairs
N_BIAS = S // TILE_P          # 4 distinct bias row-tiles
PW = 2 * D                    # 2048 cols per pair psum tile
XV = 768                      # DVE fused drain+bias covers cols [0:XV)

_NC_CACHE = None


def _build_nc():
    nc = bacc.Bacc("TRN2", target_bir_lowering=False, debug=False,
                   num_devices=N_CORES)
    # x pre-rearranged on host (layout + cast to bf16):
    # xr[p, i*F:(i+1)*F] = x[i*128+p]; cols [0:128] = A^T
    x_d = nc.declare_dram_parameter("x", [TILE_P, TILE_P + N_TILES * F],
                                    mybir.dt.bfloat16, isOutput=False)
    # combined constants [128, 5120]: cols [0:1024]=W stacked twice
    # (partitions 0-63 and 64-127 both hold W) | [1024:5120]=bias0..3
    cc_d = nc.declare_dram_parameter("cc", [TILE_P, 5 * D + TILE_P],
                                     mybir.dt.bfloat16, isOutput=False)
    out_d = nc.declare_dram_parameter("out", [ROWS, D], mybir.dt.bfloat16,
                                      isOutput=True)

    with tile.TileContext(nc) as tc:
        with (
            tc.tile_pool(name="const", bufs=1) as cpool,
            tc.tile_pool(name="outp", bufs=4) as opool,
            tc.tile_pool(name="ps", bufs=2, space="PSUM") as psp,
        ):
            # loads: sync ring carries A^T+x then the stores; scalar ring
            # carries W and the bias tiles in need-order.
            at_x0 = cpool.tile([TILE_P, 2 * TILE_P], mybir.dt.bfloat16)
            nc.sync.dma_start(at_x0[:], x_d[:, 0:2 * TILE_P])
            at_ap = at_x0[:, 0:TILE_P]
            CCW = 5 * D + TILE_P
            BB = D + TILE_P            # bias block base in cc
            cc_sb = cpool.tile([TILE_P, CCW], mybir.dt.bfloat16)
            nc.scalar.dma_start(cc_sb[:, 0:BB], cc_d[:, 0:BB])
            xr_sb = cpool.tile([TILE_P, (N_PAIR - 1) * TILE_P],
                               mybir.dt.bfloat16)
            nc.sync.dma_start(xr_sb[:, 0:7 * TILE_P],
                              x_d[:, 2 * TILE_P:9 * TILE_P])
            nc.scalar.dma_start(cc_sb[:, BB:BB + 2 * D],
                                cc_d[:, BB:BB + 2 * D])
            nc.sync.dma_start(xr_sb[:, 7 * TILE_P:15 * TILE_P],
                              x_d[:, 9 * TILE_P:17 * TILE_P])
            nc.scalar.dma_start(cc_sb[:, BB + 2 * D:CCW],
                                cc_d[:, BB + 2 * D:CCW])
            i_ap = cc_sb[:, D:D + TILE_P]

            def bias_pair(j):
                base = BB + ((2 * j) % N_BIAS) * D
                return cc_sb[:, base:base + PW]

            def w_ap(u, lo, hi):
                return cc_sb[64 * u:64 * u + F, lo:hi]

            # resident x~^T (bf16): xt_sb[64u+f, 128j+n] = x~[2j+u, n, f]
            xt_sb = cpool.tile([TILE_P, N_PAIR * TILE_P], mybir.dt.bfloat16)

            # prologue: all transpose+mix matmuls into one 4-bank psum
            # workspace; one ACT copy per 4 pairs builds xt_sb.
            ws = psp.tile([TILE_P, PW], mybir.dt.float32, name="ws",
                          tag="pair")
            for b4 in range(4):
                for k in range(4):
                    pr = 4 * b4 + k
                    src = (at_x0[:, TILE_P:2 * TILE_P] if pr == 0 else
                           xr_sb[:, 128 * (pr - 1):128 * pr])
                    nc.tensor.matmul(ws[:, 512 * b4 + 128 * k:
                                        512 * b4 + 128 * (k + 1)],
                                     src, at_ap, start=True, stop=True)
                if b4 % 2 == 0:
                    nc.scalar.copy(xt_sb[:, 512 * b4:512 * (b4 + 1)],
                                   ws[:, 512 * b4:512 * (b4 + 1)])
                else:
                    nc.vector.tensor_copy(xt_sb[:, 512 * b4:512 * (b4 + 1)],
                                          ws[:, 512 * b4:512 * (b4 + 1)])

            # steady loop, software-pipelined by one pair on DVE
            prev = None
            for j in range(N_PAIR):
                bias = bias_pair(j)
                pair = psp.tile([TILE_P, PW], mybir.dt.float32,
                                name="pair", tag="pair")
                lhs0 = xt_sb[0:64, 128 * j:128 * (j + 1)]
                lhs1 = xt_sb[64:128, 128 * j:128 * (j + 1)]
                nc.tensor.matmul(pair[:, 0:512], lhs0,
                                 w_ap(0, 0, 512), start=True, stop=True)
                nc.tensor.matmul(pair[:, 512:1024], lhs0,
                                 w_ap(0, 512, 1024), start=True, stop=True)
                nc.tensor.matmul(pair[:, 1024:1536], lhs1,
                                 w_ap(1, 0, 512), start=True, stop=True)
                nc.tensor.matmul(pair[:, 1536:2048], lhs1,
                                 w_ap(1, 512, 1024), start=True, stop=True)
                # PE injects bias into [1536:2048) (accumulate after m4;
                # same engine so the start=True reset ordering is safe)
                nc.tensor.matmul(pair[:, 1536:2048], i_ap,
                                 bias[:, 1536:2048], start=False, stop=True,
                                 skip_group_check=True)
                o_t = opool.tile([TILE_P, PW], mybir.dt.bfloat16,
                                 name="o_t")
                # DVE fused drain+bias (PSUM 1x), ACT plain drain (1x)
                nc.vector.tensor_add(o_t[:, 0:XV], pair[:, 0:XV],
                                     bias[:, 0:XV])
                nc.scalar.copy(o_t[:, XV:PW], pair[:, XV:PW])
                if prev is not None:
                    _finish_pair(nc, out_d, *prev)
                prev = (j, o_t, bias)
            _finish_pair(nc, out_d, *prev)
    nc.compile()
    return nc


def _finish_pair(nc, out_d, j, o_t, bias):
    # bias add for ACT's drained region: bf16 SBUF tensor_tensor (2x),
    # in place; then both stores.
    nc.vector.tensor_add(o_t[:, XV:1536], o_t[:, XV:1536],
                         bias[:, XV:1536])
    nc.sync.dma_start(out_d[256 * j:256 * j + 128, :], o_t[:, 0:D])
    nc.sync.dma_start(out_d[256 * j + 128:256 * j + 256, :], o_t[:, D:PW])


def _host_constants(W_emb, b_emb, w_seg, b_seg):
    # sinusoidal positional encoding, float32, same formula as the reference
    pos = np.arange(S, dtype=np.float32)[:, None]
    div = np.exp(np.arange(0, D, 2, dtype=np.float32)
                 * (-np.log(10000.0) / D)).astype(np.float32)
    ang = pos * div
    pe = np.zeros((S, D), np.float32)
    pe[:, 0::2] = np.sin(ang)
    pe[:, 1::2] = np.cos(ang)

    bias = (pe + b_emb[None, :] * (np.float32(1.0) + w_seg.sum())
            + b_seg[0]).astype(np.float32)
    # rearrange to [128, 4*D]: column block j holds bias rows j*128..j*128+127
    bias_r = np.ascontiguousarray(
        bias.reshape(N_BIAS, TILE_P, D).transpose(1, 0, 2).reshape(
            TILE_P, N_BIAS * D)).astype(ml_dtypes.bfloat16)

    blk = np.eye(SEG, dtype=np.float32) + w_seg[:, None] * np.ones(
        (1, SEG), np.float32)
    at = np.kron(np.eye(TILE_P // SEG, dtype=np.float32), blk).astype(
        ml_dtypes.bfloat16)

    wb = np.vstack([W_emb, W_emb]).astype(ml_dtypes.bfloat16)
    ident = np.eye(TILE_P, dtype=np.float32).astype(ml_dtypes.bfloat16)
    # combined consts: [W2|I128|bias0..3] as [128, 5*D+128] bf16
    cc = np.ascontiguousarray(np.concatenate([wb, ident, bias_r], axis=1))
    return at, cc


def _prepare_in_maps(x, W_emb, b_emb, w_seg, b_seg):
    x = np.ascontiguousarray(np.asarray(x, dtype=np.float32))
    W_emb = np.asarray(W_emb, dtype=np.float32)
    b_emb = np.asarray(b_emb, dtype=np.float32)
    w_seg = np.asarray(w_seg, dtype=np.float32)
    b_seg = np.asarray(b_seg, dtype=np.float32)

    at, cc = _host_constants(W_emb, b_emb, w_seg, b_seg)

    in_maps = []
    for c in range(N_CORES):
        xs = x[c * B_LOC:(c + 1) * B_LOC].reshape(ROWS, F)
        # rearrange [32 tiles, 128 rows, F] -> [128, 32*F], bf16 staging
        xr = np.ascontiguousarray(
            xs.reshape(N_TILES, TILE_P, F).transpose(1, 0, 2).reshape(
                TILE_P, N_TILES * F)).astype(ml_dtypes.bfloat16)
        in_maps.append(
            {"x": np.ascontiguousarray(np.concatenate([at, xr], axis=1)),
             "cc": cc})
    return in_maps


def kernel(x, W_emb, b_emb, w_seg, b_seg):
    in_maps = _prepare_in_maps(x, W_emb, b_emb, w_seg, b_seg)

    global _NC_CACHE
    if _NC_CACHE is None:
        _NC_CACHE = _build_nc()

    res = run_bass_kernel_spmd(_NC_CACHE, in_maps,
                               core_ids=list(range(N_CORES)))
    out = np.concatenate(
        [np.asarray(res.results[c]["out"]).astype(np.float32).reshape(
            B_LOC, S, D) for c in range(N_CORES)], axis=0)
    return out
